# revision 6
# baseline (speedup 1.0000x reference)
"""Multi-head attention (B=2, N=2048, D=1024, H=16, hd=64) on 8 trn2 NeuronCores.

Sharding: 8 cores = 2 (batch) x 4 (head groups of 4 heads).
Core c: batch b = c // 4, heads hg*4 .. hg*4+3 where hg = c % 4.

Per-core program (identical SPMD program, per-core data):
  inputs (DRAM):
    xT     [1024, 2048]  = x[b].T
    wqkT   [1024, 512]   = w_qkv[[q rows, k rows] of local heads].T
    wvT    [1024, 256]   = w_qkv[v rows of local heads].T
    wprojT [256, 1024]   = w_proj[:, local head cols].T
  output:
    out    [2048, 1024]  partial (row-parallel) projection output

  1) qkT  [512, 2048]  = wqkT.T @ xT        (q,k in transposed layout)
     v    [2048, 256]  = (xT.T @ wvT)       (natural layout, ones-augmented)
  2) per (q-block 512, head): scores_T [keys, q] = kT.T @ qT, exp on ACT
     (scale 1/8 fused), PV with ones-augmented V gives unnormalized out_T
     [64, q] + denominator row; reciprocal + K=1 outer-product matmul
     broadcast + DVE multiply normalizes.
  3) partial = attn_out_T.T @ wprojT  -> [2048, 1024]

Host unshard: out[b] = sum over 4 head-group partials + b_proj.
"""

import sys

if "/opt/trn_rl_repo" not in sys.path:
    sys.path.insert(0, "/opt/trn_rl_repo")

import numpy as np

B, N, D, H, HD = 2, 2048, 1024, 16, 64
NCORES = 8
HPC = 4               # heads per core
LQK = HPC * HD        # 256 local q (or k) rows
SCALE = HD ** -0.5    # 0.125

_CACHE = {}


def _build_program():
    import concourse.tile as tile
    from concourse import bacc, mybir

    F32 = mybir.dt.float32
    F32R = mybir.dt.float32r
    Exp = mybir.ActivationFunctionType.Exp

    nc = bacc.Bacc("TRN2", target_bir_lowering=False, debug=False,
                   num_devices=NCORES)

    xT_d = nc.dram_tensor("xT", [D, N], F32, kind="ExternalInput").ap()
    wqkT_d = nc.dram_tensor("wqkT", [D, 2 * LQK], F32, kind="ExternalInput").ap()
    wvT_d = nc.dram_tensor("wvT", [D, LQK], F32, kind="ExternalInput").ap()
    wprojT_d = nc.dram_tensor("wprojT", [LQK, D], F32, kind="ExternalInput").ap()
    out_d = nc.dram_tensor("out", [N, D], F32, kind="ExternalOutput").ap()

    KT = D // 128        # 8 contraction tiles for qkv gemms
    NB = N // 512        # 4 seq blocks
    NT = N // 128        # 16 seq tiles
    r = lambda ap: ap  # tiles already f32r

    with tile.TileContext(nc) as tc:
        with (
            nc.allow_low_precision(reason="fp32r matmul operands"),
            tc.tile_pool(name="const", bufs=1) as cpool,
            tc.tile_pool(name="w", bufs=1) as wpool,
            tc.tile_pool(name="x", bufs=1) as xpool,
            tc.tile_pool(name="qk", bufs=1) as qkpool,
            tc.tile_pool(name="vaug", bufs=1) as vapool,
            tc.tile_pool(name="ao", bufs=1) as aopool,
            tc.tile_pool(name="probs", bufs=3) as prpool,
            tc.tile_pool(name="small", bufs=2) as smpool,
            tc.tile_pool(name="stage", bufs=3) as stpool,
            tc.tile_pool(name="psbig", bufs=2, space="PSUM") as psbig,
            tc.tile_pool(name="pspv", bufs=2, space="PSUM") as pspv,
            tc.tile_pool(name="psbc", bufs=2, space="PSUM") as psbc,
        ):
            ones_f32 = cpool.tile([128, 128], F32)
            nc.vector.memset(ones_f32[:, :], 1.0)
            ones_sb = cpool.tile([65, 128], F32R)
            nc.vector.tensor_copy(ones_sb[:, :], ones_f32[0:65, :])

            # ---- input DMAs (kt-chunked so compute starts early) ----
            x_sb = xpool.tile([128, KT, N], F32R)
            wqk_sb = wpool.tile([128, KT, 2 * LQK], F32R)
            wv_sb = wpool.tile([128, KT, LQK], F32R)
            xT_r = xT_d.bitcast(F32R).rearrange("(kt p) n -> p kt n", p=128)
            wqkT_r = wqkT_d.bitcast(F32R).rearrange("(kt p) m -> p kt m", p=128)
            wvT_r = wvT_d.bitcast(F32R).rearrange("(kt p) m -> p kt m", p=128)
            for kt in range(KT):
                nc.sync.dma_start(out=wqk_sb[:, kt, :], in_=wqkT_r[:, kt, :])
                nc.sync.dma_start(out=x_sb[:, kt, :], in_=xT_r[:, kt, :])
                nc.sync.dma_start(out=wv_sb[:, kt, :], in_=wvT_r[:, kt, :])

            # ---- qkT = wqkT.T @ xT : [512, 2048], m-tiles of 128 ----
            # qk_sb m-tile layout: m=0: q heads 0,1 / m=1: q heads 2,3
            #                      m=2: k heads 0,1 / m=3: k heads 2,3
            qk_sb = qkpool.tile([128, 4, N], F32R)

            def qk_gemm(m):
                wm = 0 if m < 2 else 2 * LQK // 2   # q cols 0..255, k cols 256..511
                wcol = wm + (m % 2) * 128
                for nb in range(NB):
                    ps = psbig.tile([128, 1024], F32, tag="big")
                    for kt in range(KT):
                        nc.tensor.matmul(
                            ps[:, 0:512],
                            r(wqk_sb[:, kt, wcol:wcol + 128]),
                            r(x_sb[:, kt, nb * 512:(nb + 1) * 512]),
                            start=(kt == 0), stop=(kt == KT - 1),
                        )
                    nc.vector.tensor_copy(
                        qk_sb[:, m, nb * 512:(nb + 1) * 512], ps[:, 0:512])

            # ---- v natural [2048, 256] ones-augmented: [128, st, h, 65] ----
            v_sb = vapool.tile([128, NT, HPC, HD + 1], F32R)

            def v_gemm(st):
                ps = psbig.tile([128, 1024], F32, tag="big")
                for kt in range(KT):
                    nc.tensor.matmul(
                        ps[:, 0:LQK],
                        r(x_sb[:, kt, st * 128:(st + 1) * 128]),
                        r(wv_sb[:, kt, :]),
                        start=(kt == 0), stop=(kt == KT - 1),
                    )
                nc.vector.tensor_copy(
                    v_sb[:, st, :, 0:HD],
                    ps[:, 0:LQK].rearrange("p (h d) -> p h d", h=HPC))
                nc.vector.tensor_copy(
                    v_sb[:, st, :, HD:HD + 1],
                    ones_f32[:, 0:HPC].rearrange("p (h c) -> p h c", c=1))

            # k first (scores lhsT), then q, then v
            for m in (2, 0, 3, 1):
                qk_gemm(m)
            for st in range(NT):
                v_gemm(st)

            # late small DMA for proj weights
            wproj_sb = wpool.tile([128, 2, D], F32R)
            nc.sync.dma_start(
                out=wproj_sb[:, :, :],
                in_=wprojT_d.bitcast(F32R).rearrange("(kt p) o -> p kt o", p=128))

            # ---- attention + projection, q-block major ----
            ao_sb = aopool.tile([128, 2, N], F32R)   # proj lhsT kt0: heads 0,1; kt1: heads 2,3

            def attn(h, qb):
                pi = (h % 2) * 64
                mq, mk = h // 2, 2 + h // 2
                qT = qk_sb[pi:pi + 64, mq, qb * 512:(qb + 1) * 512]
                pv = pspv.tile([65, 512], F32, tag="pv")
                for kk in range(KT):          # pairs of key tiles
                    sc = psbig.tile([128, 1024], F32, tag="big")
                    pr = prpool.tile([128, 1024], F32R, tag="probs")
                    for j in range(2):
                        kt = 2 * kk + j
                        kT = qk_sb[pi:pi + 64, mk, kt * 128:(kt + 1) * 128]
                        nc.tensor.matmul(
                            sc[:, j * 512:(j + 1) * 512], r(kT), r(qT),
                            start=True, stop=True)
                    nc.scalar.activation(pr[:, :], sc[:, :], Exp, scale=SCALE)
                    for j in range(2):
                        kt = 2 * kk + j
                        nc.tensor.matmul(
                            pv[:, :],
                            r(v_sb[:, kt, h, 0:HD + 1]),
                            r(pr[:, j * 512:(j + 1) * 512]),
                            start=(kk == 0 and j == 0),
                            stop=(kk == KT - 1 and j == 1),
                        )
                # normalize: recip of denom row, PE outer-product broadcast,
                # DVE multiply; DMA to proj-layout partitions.
                recip = smpool.tile([65, 512], F32R, tag="recip")
                nc.vector.reciprocal(recip[64:65, :], pv[64:65, :])
                bc = psbc.tile([64, 512], F32, tag="bc")
                nc.tensor.matmul(bc[:, :], r(ones_sb[64:65, 0:64]),
                                 r(recip[64:65, :]), start=True, stop=True)
                bcs = smpool.tile([64, 512], F32, tag="bcs")
                nc.vector.tensor_copy(bcs[:, :], bc[:, :])
                aos = stpool.tile([64, 512], F32R, tag="aos")
                nc.vector.tensor_mul(aos[:, :], pv[0:64, :], bcs[:, :])
                nc.sync.dma_start(
                    out=ao_sb[pi:pi + 64, h // 2, qb * 512:(qb + 1) * 512],
                    in_=aos[:, :])

            def proj(qb):
                for nt in range(qb * 4, qb * 4 + 4):
                    outst = stpool.tile([128, D], F32, tag="outst")
                    for ob in range(2):
                        ps = psbig.tile([128, 1024], F32, tag="big")
                        for kt2 in range(2):
                            nc.tensor.matmul(
                                ps[:, 0:512],
                                r(ao_sb[:, kt2, nt * 128:(nt + 1) * 128]),
                                r(wproj_sb[:, kt2, ob * 512:(ob + 1) * 512]),
                                start=(kt2 == 0), stop=(kt2 == 1),
                            )
                        nc.vector.tensor_copy(
                            outst[:, ob * 512:(ob + 1) * 512], ps[:, 0:512])
                    nc.sync.dma_start(
                        out=out_d[nt * 128:(nt + 1) * 128, :], in_=outst[:, :])

            for qb in range(NB):
                for h in range(HPC):
                    attn(h, qb)
                proj(qb)

    nc.compile()
    return nc


def _get_program():
    if "nc" not in _CACHE:
        _CACHE["nc"] = _build_program()
    return _CACHE["nc"]


def _make_in_maps(x, w_qkv, w_proj):
    x = np.asarray(x, dtype=np.float32)
    w_qkv = np.asarray(w_qkv, dtype=np.float32)
    w_proj = np.asarray(w_proj, dtype=np.float32)
    xT = [np.ascontiguousarray(x[b].T) for b in range(B)]
    in_maps = []
    for c in range(NCORES):
        b, hg = c // 4, c % 4
        rows = slice(hg * LQK, (hg + 1) * LQK)
        qk_rows = np.r_[np.arange(hg * LQK, (hg + 1) * LQK),
                        D + np.arange(hg * LQK, (hg + 1) * LQK)]
        in_maps.append({
            "xT": xT[b],
            "wqkT": np.ascontiguousarray(w_qkv[qk_rows, :].T),
            "wvT": np.ascontiguousarray(w_qkv[2 * D + np.arange(hg * LQK, (hg + 1) * LQK), :].T),
            "wprojT": np.ascontiguousarray(w_proj[:, rows].T),
        })
    return in_maps


def kernel(x, w_qkv, w_proj, b_proj, _return_results=False, _trace=False):
    from concourse import bass_utils

    nc = _get_program()
    in_maps = _make_in_maps(x, w_qkv, w_proj)
    res = bass_utils.run_bass_kernel_spmd(
        nc, in_maps, list(range(NCORES)), trace=_trace)
    partials = np.stack([res.results[c]["out"] for c in range(NCORES)])
    out = partials.reshape(B, 4, N, D).sum(axis=1, dtype=np.float32)
    out = out + np.asarray(b_proj, dtype=np.float32)[None, None, :]
    out = out.astype(np.float32)
    if _return_results:
        return out, res
    return out


# revision 9
# speedup vs baseline: 1.1789x; 1.1789x over previous
"""Multi-head attention (B=2, N=2048, D=1024, H=16, hd=64) on 8 trn2 NeuronCores.

Sharding: 8 cores = 2 (batch) x 4 (head groups of 4 heads).
Core c: batch b = c // 4, heads hg*4 .. hg*4+3 where hg = c % 4.

Per-core program (identical SPMD program, per-core data):
  inputs (DRAM):
    xT     [1024, 2048]  = x[b].T
    wqkT   [1024, 512]   = w_qkv[[q rows, k rows] of local heads].T
    wvT    [1024, 256]   = w_qkv[v rows of local heads].T
    wprojT [256, 1024]   = w_proj[:, local head cols].T
  output:
    out    [2048, 1024]  partial (row-parallel) projection output

  1) qkT  [512, 2048]  = wqkT.T @ xT        (q,k in transposed layout)
     v    [2048, 256]  = (xT.T @ wvT)       (natural layout, ones-augmented)
  2) per (q-block 512, head): scores_T [keys, q] = kT.T @ qT, exp on ACT
     (scale 1/8 fused), PV with ones-augmented V gives unnormalized out_T
     [64, q] + denominator row; reciprocal + K=1 outer-product matmul
     broadcast + DVE multiply normalizes.
  3) partial = attn_out_T.T @ wprojT  -> [2048, 1024]

Host unshard: out[b] = sum over 4 head-group partials + b_proj.
"""

import sys

if "/opt/trn_rl_repo" not in sys.path:
    sys.path.insert(0, "/opt/trn_rl_repo")

import numpy as np

B, N, D, H, HD = 2, 2048, 1024, 16, 64
NCORES = 8
HPC = 4               # heads per core
LQK = HPC * HD        # 256 local q (or k) rows
SCALE = HD ** -0.5    # 0.125

_CACHE = {}


def _build_program():
    import concourse.tile as tile
    from concourse import bacc, mybir

    F32 = mybir.dt.float32
    F32R = mybir.dt.float32r
    BF16 = mybir.dt.bfloat16
    Exp = mybir.ActivationFunctionType.Exp

    nc = bacc.Bacc("TRN2", target_bir_lowering=False, debug=False,
                   num_devices=NCORES)

    xT_d = nc.dram_tensor("xT", [D, N], BF16, kind="ExternalInput").ap()
    wqkT_d = nc.dram_tensor("wqkT", [D, 2 * LQK], BF16, kind="ExternalInput").ap()
    wvT_d = nc.dram_tensor("wvT", [D, LQK], BF16, kind="ExternalInput").ap()
    wprojT_d = nc.dram_tensor("wprojT", [LQK, D], BF16, kind="ExternalInput").ap()
    out_d = nc.dram_tensor("out", [N, D], F32, kind="ExternalOutput").ap()

    KT = D // 128        # 8 contraction tiles for qkv gemms
    NB = N // 512        # 4 seq blocks
    NT = N // 128        # 16 seq tiles
    r = lambda ap: ap  # tiles already f32r

    with tile.TileContext(nc) as tc:
        with (
            nc.allow_low_precision(reason="fp32r matmul operands"),
            tc.tile_pool(name="const", bufs=1) as cpool,
            tc.tile_pool(name="w", bufs=1) as wpool,
            tc.tile_pool(name="x", bufs=1) as xpool,
            tc.tile_pool(name="qk", bufs=1) as qkpool,
            tc.tile_pool(name="vaug", bufs=1) as vapool,
            tc.tile_pool(name="ao", bufs=1) as aopool,
            tc.tile_pool(name="probs", bufs=3) as prpool,
            tc.tile_pool(name="small", bufs=2) as smpool,
            tc.tile_pool(name="stage", bufs=3) as stpool,
            tc.tile_pool(name="psbig", bufs=2, space="PSUM") as psbig,
            tc.tile_pool(name="pspv", bufs=2, space="PSUM") as pspv,
            tc.tile_pool(name="psbc", bufs=2, space="PSUM") as psbc,
        ):
            ones_f32 = cpool.tile([128, 128], F32)
            nc.vector.memset(ones_f32[:, :], 1.0)
            ones_sb = cpool.tile([65, 128], F32R)
            nc.vector.tensor_copy(ones_sb[:, :], ones_f32[0:65, :])

            # ---- input DMAs (kt-chunked so compute starts early) ----
            x_sb = xpool.tile([128, KT, N], BF16)
            wqk_sb = wpool.tile([128, KT, 2 * LQK], BF16)
            wv_sb = wpool.tile([128, KT, LQK], BF16)
            xT_r = xT_d.rearrange("(kt p) n -> p kt n", p=128)
            wqkT_r = wqkT_d.rearrange("(kt p) m -> p kt m", p=128)
            wvT_r = wvT_d.rearrange("(kt p) m -> p kt m", p=128)
            for kt in range(KT):
                nc.sync.dma_start(out=wqk_sb[:, kt, :], in_=wqkT_r[:, kt, :])
                nc.sync.dma_start(out=x_sb[:, kt, :], in_=xT_r[:, kt, :])
                nc.sync.dma_start(out=wv_sb[:, kt, :], in_=wvT_r[:, kt, :])

            # ---- qkT = wqkT.T @ xT : [512, 2048], m-tiles of 128 ----
            # qk_sb m-tile layout: m=0: q heads 0,1 / m=1: q heads 2,3
            #                      m=2: k heads 0,1 / m=3: k heads 2,3
            qk_sb = qkpool.tile([128, 4, N], BF16)

            def qk_gemm(m):
                wm = 0 if m < 2 else 2 * LQK // 2   # q cols 0..255, k cols 256..511
                wcol = wm + (m % 2) * 128
                for nb in range(NB):
                    ps = psbig.tile([128, 1024], F32, tag="big")
                    for kt in range(KT):
                        nc.tensor.matmul(
                            ps[:, 0:512],
                            r(wqk_sb[:, kt, wcol:wcol + 128]),
                            r(x_sb[:, kt, nb * 512:(nb + 1) * 512]),
                            start=(kt == 0), stop=(kt == KT - 1),
                        )
                    nc.vector.tensor_copy(
                        qk_sb[:, m, nb * 512:(nb + 1) * 512], ps[:, 0:512])

            # ---- v natural [2048, 256] ones-augmented: [128, st, h, 65] ----
            v_sb = vapool.tile([128, NT, HPC, HD + 1], BF16)

            def v_gemm(st):
                ps = psbig.tile([128, 1024], F32, tag="big")
                for kt in range(KT):
                    nc.tensor.matmul(
                        ps[:, 0:LQK],
                        r(x_sb[:, kt, st * 128:(st + 1) * 128]),
                        r(wv_sb[:, kt, :]),
                        start=(kt == 0), stop=(kt == KT - 1),
                    )
                nc.vector.tensor_copy(
                    v_sb[:, st, :, 0:HD],
                    ps[:, 0:LQK].rearrange("p (h d) -> p h d", h=HPC))
                nc.vector.tensor_copy(
                    v_sb[:, st, :, HD:HD + 1],
                    ones_f32[:, 0:HPC].rearrange("p (h c) -> p h c", c=1))

            # k first (scores lhsT), then q, then v
            for m in (2, 0, 3, 1):
                qk_gemm(m)
            for st in range(NT):
                v_gemm(st)

            # late small DMA for proj weights
            wproj_sb = wpool.tile([128, 2, D], BF16)
            nc.sync.dma_start(
                out=wproj_sb[:, :, :],
                in_=wprojT_d.rearrange("(kt p) o -> p kt o", p=128))

            # ---- attention + projection, q-block major ----
            ao_sb = aopool.tile([128, 2, N], BF16)   # proj lhsT kt0: heads 0,1; kt1: heads 2,3

            def attn(h, qb):
                pi = (h % 2) * 64
                mq, mk = h // 2, 2 + h // 2
                qT = qk_sb[pi:pi + 64, mq, qb * 512:(qb + 1) * 512]
                pv = pspv.tile([65, 512], F32, tag="pv")
                for kk in range(KT):          # pairs of key tiles
                    sc = psbig.tile([128, 1024], F32, tag="big")
                    pr = prpool.tile([128, 1024], BF16, tag="probs")
                    for j in range(2):
                        kt = 2 * kk + j
                        kT = qk_sb[pi:pi + 64, mk, kt * 128:(kt + 1) * 128]
                        nc.tensor.matmul(
                            sc[:, j * 512:(j + 1) * 512], r(kT), r(qT),
                            start=True, stop=True)
                    nc.scalar.activation(pr[:, :], sc[:, :], Exp, scale=SCALE)
                    for j in range(2):
                        kt = 2 * kk + j
                        nc.tensor.matmul(
                            pv[:, :],
                            r(v_sb[:, kt, h, 0:HD + 1]),
                            r(pr[:, j * 512:(j + 1) * 512]),
                            start=(kk == 0 and j == 0),
                            stop=(kk == KT - 1 and j == 1),
                        )
                # normalize: recip of denom row, PE outer-product broadcast,
                # DVE multiply; DMA to proj-layout partitions.
                recip = smpool.tile([65, 512], F32R, tag="recip")
                nc.vector.reciprocal(recip[64:65, :], pv[64:65, :])
                bc = psbc.tile([64, 512], F32, tag="bc")
                nc.tensor.matmul(bc[:, :], r(ones_sb[64:65, 0:64]),
                                 r(recip[64:65, :]), start=True, stop=True)
                bcs = smpool.tile([64, 512], F32, tag="bcs")
                nc.vector.tensor_copy(bcs[:, :], bc[:, :])
                aos = stpool.tile([64, 512], BF16, tag="aos")
                nc.vector.tensor_mul(aos[:, :], pv[0:64, :], bcs[:, :])
                nc.sync.dma_start(
                    out=ao_sb[pi:pi + 64, h // 2, qb * 512:(qb + 1) * 512],
                    in_=aos[:, :])

            def proj(qb):
                for nt in range(qb * 4, qb * 4 + 4):
                    outst = stpool.tile([128, D], F32, tag="outst")
                    for ob in range(2):
                        ps = psbig.tile([128, 1024], F32, tag="big")
                        for kt2 in range(2):
                            nc.tensor.matmul(
                                ps[:, 0:512],
                                r(ao_sb[:, kt2, nt * 128:(nt + 1) * 128]),
                                r(wproj_sb[:, kt2, ob * 512:(ob + 1) * 512]),
                                start=(kt2 == 0), stop=(kt2 == 1),
                            )
                        nc.vector.tensor_copy(
                            outst[:, ob * 512:(ob + 1) * 512], ps[:, 0:512])
                    nc.sync.dma_start(
                        out=out_d[nt * 128:(nt + 1) * 128, :], in_=outst[:, :])

            for qb in range(NB):
                for h in range(HPC):
                    attn(h, qb)
                proj(qb)

    nc.compile()
    return nc


def _get_program():
    if "nc" not in _CACHE:
        _CACHE["nc"] = _build_program()
    return _CACHE["nc"]


def _make_in_maps(x, w_qkv, w_proj):
    import ml_dtypes
    bf16 = ml_dtypes.bfloat16
    x = np.asarray(x, dtype=np.float32)
    w_qkv = np.asarray(w_qkv, dtype=np.float32)
    w_proj = np.asarray(w_proj, dtype=np.float32)
    xT = [np.ascontiguousarray(x[b].T).astype(bf16) for b in range(B)]
    in_maps = []
    for c in range(NCORES):
        b, hg = c // 4, c % 4
        rows = slice(hg * LQK, (hg + 1) * LQK)
        qk_rows = np.r_[np.arange(hg * LQK, (hg + 1) * LQK),
                        D + np.arange(hg * LQK, (hg + 1) * LQK)]
        in_maps.append({
            "xT": xT[b],
            "wqkT": np.ascontiguousarray(w_qkv[qk_rows, :].T).astype(bf16),
            "wvT": np.ascontiguousarray(
                w_qkv[2 * D + np.arange(hg * LQK, (hg + 1) * LQK), :].T).astype(bf16),
            "wprojT": np.ascontiguousarray(w_proj[:, rows].T).astype(bf16),
        })
    return in_maps


def kernel(x, w_qkv, w_proj, b_proj, _return_results=False, _trace=False):
    from concourse import bass_utils

    nc = _get_program()
    in_maps = _make_in_maps(x, w_qkv, w_proj)
    res = bass_utils.run_bass_kernel_spmd(
        nc, in_maps, list(range(NCORES)), trace=_trace)
    partials = np.stack([res.results[c]["out"] for c in range(NCORES)])
    out = partials.reshape(B, 4, N, D).sum(axis=1, dtype=np.float32)
    out = out + np.asarray(b_proj, dtype=np.float32)[None, None, :]
    out = out.astype(np.float32)
    if _return_results:
        return out, res
    return out


# revision 10
# speedup vs baseline: 1.5196x; 1.2890x over previous
"""Multi-head attention (B=2, N=2048, D=1024, H=16, hd=64) on 8 trn2 NeuronCores.

Sharding: 8 cores = 2 (batch) x 4 (head groups of 4 heads).
Core c: batch b = c // 4, heads hg*4 .. hg*4+3 where hg = c % 4.

Per-core program (identical SPMD program, per-core data):
  inputs (DRAM):
    xT     [1024, 2048]  = x[b].T
    wqkT   [1024, 512]   = w_qkv[[q rows, k rows] of local heads].T
    wvT    [1024, 256]   = w_qkv[v rows of local heads].T
    wprojT [256, 1024]   = w_proj[:, local head cols].T
  output:
    out    [2048, 1024]  partial (row-parallel) projection output

  1) qkT  [512, 2048]  = wqkT.T @ xT        (q,k in transposed layout)
     v    [2048, 256]  = (xT.T @ wvT)       (natural layout, ones-augmented)
  2) per (q-block 512, head): scores_T [keys, q] = kT.T @ qT, exp on ACT
     (scale 1/8 fused), PV with ones-augmented V gives unnormalized out_T
     [64, q] + denominator row; reciprocal + K=1 outer-product matmul
     broadcast + DVE multiply normalizes.
  3) partial = attn_out_T.T @ wprojT  -> [2048, 1024]

Host unshard: out[b] = sum over 4 head-group partials + b_proj.
"""

import sys

if "/opt/trn_rl_repo" not in sys.path:
    sys.path.insert(0, "/opt/trn_rl_repo")

import numpy as np

B, N, D, H, HD = 2, 2048, 1024, 16, 64
NCORES = 8
HPC = 4               # heads per core
LQK = HPC * HD        # 256 local q (or k) rows
SCALE = HD ** -0.5    # 0.125

_CACHE = {}


def _build_program():
    import concourse.tile as tile
    from concourse import bacc, mybir

    F32 = mybir.dt.float32
    F32R = mybir.dt.float32r
    BF16 = mybir.dt.bfloat16
    Exp = mybir.ActivationFunctionType.Exp

    nc = bacc.Bacc("TRN2", target_bir_lowering=False, debug=False,
                   num_devices=NCORES)

    xT_d = nc.dram_tensor("xT", [D, N], BF16, kind="ExternalInput").ap()
    wqkT_d = nc.dram_tensor("wqkT", [D, 2 * LQK], BF16, kind="ExternalInput").ap()
    wvT_d = nc.dram_tensor("wvT", [D, LQK], BF16, kind="ExternalInput").ap()
    wprojT_d = nc.dram_tensor("wprojT", [LQK, D], BF16, kind="ExternalInput").ap()
    out_d = nc.dram_tensor("out", [N, D], F32, kind="ExternalOutput").ap()

    KT = D // 128        # 8 contraction tiles for qkv gemms
    NB = N // 512        # 4 seq blocks
    NT = N // 128        # 16 seq tiles
    r = lambda ap: ap  # tiles already f32r

    with tile.TileContext(nc) as tc:
        with (
            nc.allow_low_precision(reason="fp32r matmul operands"),
            tc.tile_pool(name="const", bufs=1) as cpool,
            tc.tile_pool(name="w", bufs=1) as wpool,
            tc.tile_pool(name="x", bufs=1) as xpool,
            tc.tile_pool(name="qk", bufs=1) as qkpool,
            tc.tile_pool(name="vaug", bufs=1) as vapool,
            tc.tile_pool(name="ao", bufs=1) as aopool,
            tc.tile_pool(name="probs", bufs=3) as prpool,
            tc.tile_pool(name="small", bufs=5) as smpool,
            tc.tile_pool(name="stage", bufs=3) as stpool,
            tc.tile_pool(name="psbig", bufs=2, space="PSUM") as psbig,
            tc.tile_pool(name="pspv", bufs=4, space="PSUM") as pspv,
        ):
            ones_f32 = cpool.tile([128, 128], F32)
            nc.vector.memset(ones_f32[:, :], 1.0)
            ones_sb = cpool.tile([65, 128], F32R)
            nc.vector.tensor_copy(ones_sb[:, :], ones_f32[0:65, :])

            # ---- input DMAs (kt-chunked so compute starts early) ----
            x_sb = xpool.tile([128, KT, N], BF16)
            wqk_sb = wpool.tile([128, KT, 2 * LQK], BF16)
            wv_sb = wpool.tile([128, KT, LQK], BF16)
            xT_r = xT_d.rearrange("(kt p) n -> p kt n", p=128)
            wqkT_r = wqkT_d.rearrange("(kt p) m -> p kt m", p=128)
            wvT_r = wvT_d.rearrange("(kt p) m -> p kt m", p=128)
            for kt in range(KT):
                nc.sync.dma_start(out=wqk_sb[:, kt, :], in_=wqkT_r[:, kt, :])
                nc.sync.dma_start(out=x_sb[:, kt, :], in_=xT_r[:, kt, :])
                nc.sync.dma_start(out=wv_sb[:, kt, :], in_=wvT_r[:, kt, :])

            # ---- qkT = wqkT.T @ xT : [512, 2048], m-tiles of 128 ----
            # qk_sb m-tile layout: m=0: q heads 0,1 / m=1: q heads 2,3
            #                      m=2: k heads 0,1 / m=3: k heads 2,3
            qk_sb = qkpool.tile([128, 4, N], BF16)

            def qk_gemm(m):
                wm = 0 if m < 2 else 2 * LQK // 2   # q cols 0..255, k cols 256..511
                wcol = wm + (m % 2) * 128
                for nb in range(NB):
                    ps = psbig.tile([128, 1024], F32, tag="big")
                    for kt in range(KT):
                        nc.tensor.matmul(
                            ps[:, 0:512],
                            r(wqk_sb[:, kt, wcol:wcol + 128]),
                            r(x_sb[:, kt, nb * 512:(nb + 1) * 512]),
                            start=(kt == 0), stop=(kt == KT - 1),
                        )
                    nc.vector.tensor_copy(
                        qk_sb[:, m, nb * 512:(nb + 1) * 512], ps[:, 0:512])

            # ---- v natural [2048, 256] ones-augmented: [128, st, h, 65] ----
            v_sb = vapool.tile([128, NT, HPC, HD + 1], BF16)

            def v_gemm(st):
                ps = psbig.tile([128, 1024], F32, tag="big")
                for kt in range(KT):
                    nc.tensor.matmul(
                        ps[:, 0:LQK],
                        r(x_sb[:, kt, st * 128:(st + 1) * 128]),
                        r(wv_sb[:, kt, :]),
                        start=(kt == 0), stop=(kt == KT - 1),
                    )
                nc.vector.tensor_copy(
                    v_sb[:, st, :, 0:HD],
                    ps[:, 0:LQK].rearrange("p (h d) -> p h d", h=HPC))
                nc.vector.tensor_copy(
                    v_sb[:, st, :, HD:HD + 1],
                    ones_f32[:, 0:HPC].rearrange("p (h c) -> p h c", c=1))

            # deps of attention (h0/h1, qb0) first so ACT starts early:
            # k tiles for heads 0/1, q tiles for heads 0/1, v, then the rest
            for m in (2, 0):
                qk_gemm(m)
            for st in range(NT):
                v_gemm(st)
            for m in (3, 1):
                qk_gemm(m)

            # late small DMA for proj weights
            wproj_sb = wpool.tile([128, 2, D], BF16)
            nc.sync.dma_start(
                out=wproj_sb[:, :, :],
                in_=wprojT_d.rearrange("(kt p) o -> p kt o", p=128))

            # ---- attention + projection, q-block major ----
            ao_sb = aopool.tile([128, 2, N], BF16)   # proj lhsT kt0: heads 0,1; kt1: heads 2,3

            def attn_chain(h, qb):
                """Scores -> exp -> PV accumulation; recip emitted inline so the
                DVE computes it while the PE runs the next head's chain. Returns
                (pv psum, recip tile) for the deferred normalize tail."""
                pi = (h % 2) * 64
                mq, mk = h // 2, 2 + h // 2
                qT = qk_sb[pi:pi + 64, mq, qb * 512:(qb + 1) * 512]
                pv = pspv.tile([65, 512], F32, tag="pv")
                for kk in range(KT):          # pairs of key tiles
                    sc = psbig.tile([128, 1024], F32, tag="big")
                    pr = prpool.tile([128, 1024], BF16, tag="probs")
                    for j in range(2):
                        kt = 2 * kk + j
                        kT = qk_sb[pi:pi + 64, mk, kt * 128:(kt + 1) * 128]
                        nc.tensor.matmul(
                            sc[:, j * 512:(j + 1) * 512], r(kT), r(qT),
                            start=True, stop=True)
                    nc.scalar.activation(pr[:, :], sc[:, :], Exp, scale=SCALE)
                    for j in range(2):
                        kt = 2 * kk + j
                        nc.tensor.matmul(
                            pv[:, :],
                            r(v_sb[:, kt, h, 0:HD + 1]),
                            r(pr[:, j * 512:(j + 1) * 512]),
                            start=(kk == 0 and j == 0),
                            stop=(kk == KT - 1 and j == 1),
                        )
                recip = smpool.tile([65, 512], F32R, tag="recip")
                nc.vector.reciprocal(recip[64:65, :], pv[64:65, :])
                return pv, recip

            def norm_tail(h, qb, pv, recip):
                """PE outer-product broadcast of recip, DVE multiply, DMA to
                the proj-layout partitions of ao_sb."""
                pi = (h % 2) * 64
                bc = psbig.tile([64, 512], F32, tag="big")
                nc.tensor.matmul(bc[:, :], r(ones_sb[64:65, 0:64]),
                                 r(recip[64:65, :]), start=True, stop=True)
                bcs = smpool.tile([64, 512], F32, tag="bcs")
                nc.vector.tensor_copy(bcs[:, :], bc[:, :])
                aos = stpool.tile([64, 512], BF16, tag="aos")
                nc.vector.tensor_mul(aos[:, :], pv[0:64, :], bcs[:, :])
                nc.sync.dma_start(
                    out=ao_sb[pi:pi + 64, h // 2, qb * 512:(qb + 1) * 512],
                    in_=aos[:, :])

            def proj(qb):
                for nt in range(qb * 4, qb * 4 + 4):
                    outst = stpool.tile([128, D], F32, tag="outst")
                    for ob in range(2):
                        ps = psbig.tile([128, 1024], F32, tag="big")
                        for kt2 in range(2):
                            nc.tensor.matmul(
                                ps[:, 0:512],
                                r(ao_sb[:, kt2, nt * 128:(nt + 1) * 128]),
                                r(wproj_sb[:, kt2, ob * 512:(ob + 1) * 512]),
                                start=(kt2 == 0), stop=(kt2 == 1),
                            )
                        nc.vector.tensor_copy(
                            outst[:, ob * 512:(ob + 1) * 512], ps[:, 0:512])
                    nc.sync.dma_start(
                        out=out_d[nt * 128:(nt + 1) * 128, :], in_=outst[:, :])

            for qb in range(NB):
                pend = [attn_chain(h, qb) for h in range(HPC)]
                for h in range(HPC):
                    norm_tail(h, qb, *pend[h])
                proj(qb)

    nc.compile()
    return nc


def _get_program():
    if "nc" not in _CACHE:
        _CACHE["nc"] = _build_program()
    return _CACHE["nc"]


def _make_in_maps(x, w_qkv, w_proj):
    import ml_dtypes
    bf16 = ml_dtypes.bfloat16
    x = np.asarray(x, dtype=np.float32)
    w_qkv = np.asarray(w_qkv, dtype=np.float32)
    w_proj = np.asarray(w_proj, dtype=np.float32)
    xT = [np.ascontiguousarray(x[b].T).astype(bf16) for b in range(B)]
    in_maps = []
    for c in range(NCORES):
        b, hg = c // 4, c % 4
        rows = slice(hg * LQK, (hg + 1) * LQK)
        qk_rows = np.r_[np.arange(hg * LQK, (hg + 1) * LQK),
                        D + np.arange(hg * LQK, (hg + 1) * LQK)]
        in_maps.append({
            "xT": xT[b],
            "wqkT": np.ascontiguousarray(w_qkv[qk_rows, :].T).astype(bf16),
            "wvT": np.ascontiguousarray(
                w_qkv[2 * D + np.arange(hg * LQK, (hg + 1) * LQK), :].T).astype(bf16),
            "wprojT": np.ascontiguousarray(w_proj[:, rows].T).astype(bf16),
        })
    return in_maps


def kernel(x, w_qkv, w_proj, b_proj, _return_results=False, _trace=False):
    from concourse import bass_utils

    nc = _get_program()
    in_maps = _make_in_maps(x, w_qkv, w_proj)
    res = bass_utils.run_bass_kernel_spmd(
        nc, in_maps, list(range(NCORES)), trace=_trace)
    partials = np.stack([res.results[c]["out"] for c in range(NCORES)])
    out = partials.reshape(B, 4, N, D).sum(axis=1, dtype=np.float32)
    out = out + np.asarray(b_proj, dtype=np.float32)[None, None, :]
    out = out.astype(np.float32)
    if _return_results:
        return out, res
    return out
